# revision 62
# baseline (speedup 1.0000x reference)
"""Trainium2 Bass kernel for nn_CDECF (graph-ODE collaborative filtering).

Contract: kernel(**inputs) takes FULL unsharded numpy inputs (as produced by
reference.setup_inputs()) and returns the FULL [8192] float32 output.

Strategy
--------
The reference scatters the 8192 batch-user embeddings into node rows [0,B)
and batch-item embeddings into rows [NU, NU+B) of a 70000-row node tensor,
runs segment_sum over 2M edges, and reads back only those same rows. Hence
only edges with BOTH endpoints inside those two windows contribute; host
preprocessing compacts the problem to a 16384-row space (~134k edges).

Compact row layout: core c owns user rows for batch b in [1024c, 1024c+1024)
followed by the matching item rows, so the MLP gate is purely core-local.

The published table packs each batch pair into one 256B row:
row b = [user_b (64 bf16) | item_b (64 bf16)]. The graph is bipartite, so a
user-row output panel only ever reads item rows and vice versa — the scatter
matmul statically selects the needed half of each gathered element, and the
256B dma_gather granularity carries zero padding. One 256KB-in AllGather per
step boundary republishes the table.

Per ODE step, per core:
  - dma_gather the referenced pair rows (deduped per output panel; call-tail
    padding is skipped via negative indices + per-core runtime counts)
  - scatter-add via one-hot selection-matrix matmuls on the TensorEngine,
    K chunks of <=128 slots per 128-row output panel, accumulated in PSUM
  - gate MLP computed from the core-local fp32 slice (overlaps the gather)
  - Euler update in fp32; the pair-packed bf16 stage republishes via
    AllGather
Final scoring (sigmoid of U.I) is a trivial host-side epilogue.
"""
import sys

for _p in ("/opt/trn_rl_repo", "/root/.axon_site/_ro/trn_rl_repo"):
    if _p not in sys.path:
        sys.path.append(_p)

import numpy as np
import ml_dtypes

import concourse.bass as bass
import concourse.bacc as bacc
import concourse.mybir as mybir
import concourse.tile as tile
from concourse import bass_utils
from concourse.masks import make_identity

BF16 = ml_dtypes.bfloat16

NCORES = 8
NU, NI, B, D = 50000, 20000, 8192, 64
ROWS = 2 * B          # 16384 compact rows
SLICE = ROWS // NCORES  # 2048 rows per core
HALF = SLICE // 2     # 1024 batch pairs per core
PAIRS = NCORES * HALF  # 8192 pair rows in the published table
PANEL = 128
NPANEL = SLICE // PANEL  # 16 panels per core
HPANEL = NPANEL // 2     # 8 panels per half
CHUNK = 128           # slots per scatter matmul
GCALL = 1024          # gather idxs per dma_gather call (descriptor-ring cap)
NSTEP = 3

_PROG_CACHE = {}


# ----------------------------------------------------------------------------
# Host preprocessing
# ----------------------------------------------------------------------------

def _compact_rows_user(b, pos):
    return SLICE * (b // HALF) + pos[b]


def _compact_rows_item(b, pos):
    return SLICE * (b // HALF) + HALF + pos[b]


def _balance_pos(edge_src, edge_dst):
    """batch index -> row position within its core half (panel*128 + i).

    Shared by the user and item halves (the published table pairs batch b's
    user and item rows at one position). Chosen by a dedup-aware greedy so
    every output panel's distinct-source count stays under 8*128, i.e. the
    scatter needs K=8 chunks per panel instead of 9 (~11% less gather
    traffic, and 16 gather calls = 4 even SWDGE queue rounds).
    """
    src = np.asarray(edge_src).astype(np.int64)
    dst = np.asarray(edge_dst).astype(np.int64)

    def in_s(x):
        return (x < B) | ((x >= NU) & (x < NU + B))

    mask = in_s(src) & in_s(dst)
    s, d = src[mask], dst[mask]
    ob = np.where(s < B, s, s - NU)
    sb = np.where(d < B, d, d - NU)
    rowid = np.where(s < B, ob, B + ob)
    order = np.argsort(rowid * B + sb)
    r_s, s_s = rowid[order], sb[order]
    keep = np.ones(len(r_s), bool)
    keep[1:] = (r_s[1:] != r_s[:-1]) | (s_s[1:] != s_s[:-1])
    r_s, s_s = r_s[keep], s_s[keep]
    starts = np.searchsorted(r_s, np.arange(2 * B + 1))

    pos = np.zeros(B, np.int64)
    for c in range(NCORES):
        batches = list(range(c * HALF, (c + 1) * HALF))
        US = {b: s_s[starts[b]:starts[b + 1]].tolist() for b in batches}
        IS = {b: s_s[starts[B + b]:starts[B + b + 1]].tolist()
              for b in batches}
        batches.sort(key=lambda b: -(len(US[b]) + len(IS[b])))
        pu = [set() for _ in range(HPANEL)]
        pi = [set() for _ in range(HPANEL)]
        pc = [0] * HPANEL
        plist = [[] for _ in range(HPANEL)]
        for b in batches:
            su, si = US[b], IS[b]
            best, bestsc = None, None
            for p in range(HPANEL):
                if pc[p] >= PANEL:
                    continue
                ru = len(pu[p]) + sum(1 for x in su if x not in pu[p])
                ri = len(pi[p]) + sum(1 for x in si if x not in pi[p])
                sc = (max(ru, ri), ru + ri)
                if bestsc is None or sc < bestsc:
                    best, bestsc = p, sc
            pu[best].update(su)
            pi[best].update(si)
            pc[best] += 1
            plist[best].append(b)
        for p in range(HPANEL):
            for i, b in enumerate(plist[p]):
                pos[b] = p * PANEL + i
    return pos


def _call_plan(K):
    """Gather call plan: list of (size_slots, [(panel, k), ...]) per call.

    Every call holds consecutive chunks of ONE panel in k order, so a panel's
    partially-filled last chunk is always at a call tail (negative-index
    padding skip works). Per phase the first 4 panels ship as one full call
    each (first queue round); the last 4 panels are split into two half-size
    calls each (two short rounds), halving both the final DMA-drain exposure
    and the after-landing matmul burst ahead of the publish.

    Also returns gci[(p, k)] -> (call g, chunk-within-call ci).
    """
    calls = []
    gci = np.zeros((NPANEL, K, 2), np.int64)
    kh = K - K // 2                   # chunks in the first half-call
    whole = K * CHUNK <= GCALL        # whole-panel calls fit the ring cap
    for ph in range(2):
        panels = list(range(ph * HPANEL, (ph + 1) * HPANEL))
        for p in panels[:4]:
            if whole:
                calls.append((K * CHUNK, [(p, k) for k in range(K)]))
            else:
                calls.append((kh * CHUNK, [(p, k) for k in range(kh)]))
                calls.append(((K - kh) * CHUNK,
                              [(p, k) for k in range(kh, K)]))
        for p in panels[4:]:
            calls.append((kh * CHUNK, [(p, k) for k in range(kh)]))
            calls.append(((K - kh) * CHUNK, [(p, k) for k in range(kh, K)]))
    for g, (size, chunks) in enumerate(calls):
        assert size <= GCALL and size % CHUNK == 0
        for ci, (p, k) in enumerate(chunks):
            gci[p, k] = (g, ci)
    return calls, gci


def _preprocess_edges(edge_src, edge_dst, edge_vals, pos):
    src = np.asarray(edge_src).astype(np.int64)
    dst = np.asarray(edge_dst).astype(np.int64)
    val = np.asarray(edge_vals).astype(np.float32)

    def in_s(x):
        return (x < B) | ((x >= NU) & (x < NU + B))

    mask = in_s(src) & in_s(dst)
    s, d, v = src[mask], dst[mask], val[mask]

    def compact(ids):
        b = np.where(ids < B, ids, ids - NU)
        return (SLICE * (b // HALF) + pos[b]
                + np.where(ids < B, 0, HALF)).astype(np.int64)

    cs, cd = compact(s), compact(d)

    # bipartite invariant: user-row outputs read item rows and vice versa
    assert np.all(((cs % SLICE) < HALF) == ((cd % SLICE) >= HALF))

    pg = cs // PANEL                        # output panel 0..127
    pr = (cd // SLICE) * HALF + (cd % SLICE) % HALF   # source pair id
    rloc = cs % PANEL                       # row within output panel

    # dedup (panel, pair) -> one gather slot; multi-hot sel absorbs repeats
    skey = pg * PAIRS + pr
    uq, inv = np.unique(skey, return_inverse=True)
    upg = uq // PAIRS
    upr = uq % PAIRS
    counts = np.bincount(upg, minlength=ROWS // PANEL)
    K = int(np.ceil(counts.max() / CHUNK))
    nchunk = NPANEL * K
    nslots = nchunk * CHUNK

    base = np.zeros(ROWS // PANEL, np.int64)
    base[1:] = np.cumsum(counts)[:-1]
    urank = np.arange(len(uq)) - base[upg]  # rank within panel (pr-sorted)
    ucore = upg // NPANEL
    up = upg % NPANEL
    k_u = urank // CHUNK
    pos_u = urank % CHUNK

    calls, gci = _call_plan(K)
    ncall = len(calls)
    off = np.zeros(ncall + 1, np.int64)
    off[1:] = np.cumsum([sz for sz, _ in calls])
    assert off[ncall] == nslots
    g_u = gci[up, k_u, 0]
    ci_u = gci[up, k_u, 1]
    slot_u = off[g_u] + ci_u * CHUNK + pos_u

    idx_arr = np.full((NCORES, nslots), -1, np.int16)
    idx_arr[ucore, slot_u] = upr.astype(np.int16)

    sel = np.zeros((NCORES, nslots, PANEL), np.float32)
    np.add.at(sel, (ucore[inv], slot_u[inv], rloc), v)
    # SBUF layout [core, 128 slot-partitions, nchunk*128 row-cols], with
    # columns ordered by linear chunk id c_lin = g*cpc + ci
    sel = sel.reshape(NCORES, nchunk, CHUNK, PANEL).transpose(0, 2, 1, 3)
    sel = np.ascontiguousarray(sel.reshape(NCORES, CHUNK, nchunk * PANEL))
    sel_bf = sel.astype(BF16)

    # per-(core, call): negatives are only allowed at the call tail; convert
    # interior holes to 0 (harmless: sel there is 0). The per-call slot count
    # must be identical across cores (it is baked into the SPMD program), so
    # pad every core up to the max core's count with index 0 and put -1 only
    # beyond it.
    nreal = np.zeros((NCORES, ncall), np.int32)
    for c in range(NCORES):
        for g in range(ncall):
            blk = idx_arr[c, off[g]:off[g + 1]]
            real = np.nonzero(blk >= 0)[0]
            last = real[-1]
            blk[:last + 1][blk[:last + 1] < 0] = 0
            nreal[c, g] = np.count_nonzero(blk >= 0)
    ncnt = nreal.max(axis=0).astype(np.int64)   # per-call count, all cores
    for c in range(NCORES):
        for g in range(ncall):
            blk = idx_arr[c, off[g]:off[g + 1]]
            blk[nreal[c, g]:ncnt[g]] = 0

    # wrapped gather indices: per call block, wrapped into 16 partitions:
    # wrapped[p, s] = block_idx[s*16 + p]
    gidx = np.zeros((NCORES, 16, nslots // 16), np.int16)
    for g in range(ncall):
        sz = off[g + 1] - off[g]
        blk = idx_arr[:, off[g]:off[g + 1]].reshape(NCORES, sz // 16, 16)
        gidx[:, :, off[g] // 16:off[g + 1] // 16] = blk.transpose(0, 2, 1)

    return K, nchunk, nslots, sel_bf, gidx, tuple(int(x) for x in ncnt)


def _slice_layout(slice_2d):
    """[2048, 64] -> SBUF layout [128, 16*64] (partition = row-in-panel)."""
    return np.ascontiguousarray(
        slice_2d.reshape(NPANEL, PANEL, D).transpose(1, 0, 2).reshape(PANEL,
                                                                      NPANEL * D))


def _unslice_layout(arr):
    """[128, 16*64] -> [2048, 64]."""
    return arr.reshape(PANEL, NPANEL, D).transpose(1, 0, 2).reshape(SLICE, D)


# ----------------------------------------------------------------------------
# Device program
# ----------------------------------------------------------------------------

def _build_program(K, nchunk, nslots, dts, ncnt):
    FP32 = mybir.dt.float32
    BF = mybir.dt.bfloat16
    nc = bacc.Bacc("TRN2", target_bir_lowering=False, debug=False,
                   num_devices=NCORES, num_swdge_queues=4)

    calls, gci = _call_plan(K)
    ncall = len(calls)
    off = [0]
    for sz, _ in calls:
        off.append(off[-1] + sz)

    # --- I/O -----------------------------------------------------------------
    table0 = nc.dram_tensor("table0", [PAIRS, 2 * D], BF, kind="ExternalInput")
    slice0 = nc.dram_tensor("slice0", [PANEL, NPANEL * D], FP32,
                            kind="ExternalInput")
    selm_in = nc.dram_tensor("selm", [PANEL, nchunk * PANEL], BF,
                             kind="ExternalInput")
    gidx_in = nc.dram_tensor("gidx", [128, nslots // 16], mybir.dt.int16,
                             kind="ExternalInput")
    w1u_in = nc.dram_tensor("w1u", [D, D], BF, kind="ExternalInput")
    w1i_in = nc.dram_tensor("w1i", [D, D], BF, kind="ExternalInput")
    w2_in = nc.dram_tensor("w2", [D, D], BF, kind="ExternalInput")
    b1_in = nc.dram_tensor("b1", [D, 1], FP32, kind="ExternalInput")
    b2_in = nc.dram_tensor("b2", [D, 1], FP32, kind="ExternalInput")
    outsl = nc.dram_tensor("outslice", [PANEL, NPANEL * D], FP32,
                           kind="ExternalOutput")

    # --- internal DRAM -------------------------------------------------------
    ag_in = [nc.dram_tensor(f"ag_in{s}", [HALF, 2 * D], BF)
             for s in range(NSTEP - 1)]
    tbl_ag = [nc.dram_tensor(f"tbl_ag{s}", [PAIRS, 2 * D], BF,
                             addr_space="Shared") for s in range(NSTEP - 1)]
    warm_in = nc.dram_tensor("warm_in", [16, 16], BF)
    warm_out = nc.dram_tensor("warm_out", [128, 16], BF, addr_space="Shared")

    with tile.TileContext(nc) as tc:
        with (
            tc.tile_pool(name="cst", bufs=1) as cst,
            tc.tile_pool(name="state", bufs=1) as state,
            tc.tile_pool(name="work", bufs=2) as work,
            tc.tile_pool(name="psum", bufs=2, space="PSUM") as psum,
            tc.tile_pool(name="psum_s", bufs=3, space="PSUM") as psum_s,
        ):
            # --- persistent tiles -------------------------------------------
            selm = cst.tile([PANEL, nchunk * PANEL], BF)
            gidx = cst.tile([128, nslots // 16], mybir.dt.int16)
            w1u = cst.tile([D, D], BF)
            w1i = cst.tile([D, D], BF)
            w2 = cst.tile([D, D], BF)
            b1 = cst.tile([D, 1], FP32)
            b2 = cst.tile([D, 1], FP32)
            ident = cst.tile([PANEL, PANEL], FP32)
            T = [state.tile([PANEL, NPANEL * D], FP32, name=f"T{i}")
                 for i in range(2)]
            G = [state.tile([PANEL, calls[g][0]], BF, name=f"G{g}")
                 for g in range(ncall)]
            agstage = state.tile([PANEL, HPANEL * 2 * D], BF)
            xTu = state.tile([D, HPANEL * PANEL], BF)
            xTi = state.tile([D, HPANEL * PANEL], BF)
            hT = state.tile([D, HPANEL * PANEL], BF)
            wT = state.tile([D, HPANEL * PANEL], FP32)
            dtw = state.tile([PANEL, HPANEL * D], FP32)
            Ysb = state.tile([PANEL, NPANEL * D], FP32)

            # gidx first: the step-0 gathers wait only on it (table0 is an
            # ExternalInput already resident in HBM); selm isn't needed until
            # the first scatter, so it loads behind the gathers.
            nc.sync.dma_start(gidx[:], gidx_in[:])
            nc.sync.dma_start(w1u[:], w1u_in[:])
            nc.sync.dma_start(w1i[:], w1i_in[:])
            nc.sync.dma_start(w2[:], w2_in[:])
            nc.sync.dma_start(b1[:], b1_in[:])
            nc.sync.dma_start(b2[:], b2_in[:])
            nc.sync.dma_start(T[0][:], slice0[:])
            nc.sync.dma_start(selm[:], selm_in[:])
            nc.vector.memset(agstage[:], 0.0)
            # skipped call-tail slots leave G untouched; zero once so the
            # (sel=0-masked) stale columns are finite
            for g in range(ncall):
                nc.vector.memset(G[g][:], 0.0)
            make_identity(nc, ident[:])

            def emit_warmup():
                # warm the ncfw collective path under the step-0 gathers so
                # the first real AllGather doesn't pay the cold trigger delay
                nc.gpsimd.collective_compute(
                    "AllGather",
                    mybir.AluOpType.bypass,
                    replica_groups=[list(range(NCORES))],
                    ins=[warm_in.ap().opt()],
                    outs=[warm_out.ap().opt()],
                )

            emit_warmup()

            for step in range(NSTEP):
                dt = float(dts[step])
                Tcur = T[step % 2]
                Tnxt = T[(step + 1) % 2]
                tbl = table0 if step == 0 else tbl_ag[step - 1]

                # ---- gather referenced pair rows ---------------------------
                for g in range(ncall):
                    sz = calls[g][0]
                    nc.gpsimd.dma_gather(
                        out_ap=G[g][:].rearrange("p (c e) -> p c e", e=2 * D),
                        in_ap=tbl.ap(),
                        idxs_ap=gidx[:, off[g] // 16:off[g + 1] // 16],
                        num_idxs=sz,
                        num_idxs_reg=int(ncnt[g]),
                        elem_size=2 * D,
                        queue_num=g % 4,
                    )

                # ---- gate MLP from local fp32 slice (core-local) -----------
                for p in range(NPANEL):
                    tp = psum.tile([D, PANEL], FP32, tag="tps")
                    nc.tensor.transpose(tp[:], Tcur[:, p * D:(p + 1) * D],
                                        ident[:])
                    dst = xTu if p < HPANEL else xTi
                    q = p % HPANEL
                    nc.scalar.copy(dst[:, q * PANEL:(q + 1) * PANEL], tp[:])
                for chix in range(2):
                    sl = slice(chix * 512, (chix + 1) * 512)
                    hp = psum.tile([D, 512], FP32, tag="mlp")
                    nc.tensor.matmul(hp[:], w1u[:], xTu[:, sl],
                                     start=True, stop=False)
                    nc.tensor.matmul(hp[:], w1i[:], xTi[:, sl],
                                     start=False, stop=True)
                    nc.scalar.activation(hT[:, sl], hp[:],
                                         mybir.ActivationFunctionType.Relu,
                                         bias=b1[:])
                    zp = psum.tile([D, 512], FP32, tag="mlp")
                    nc.tensor.matmul(zp[:], w2[:], hT[:, sl],
                                     start=True, stop=True)
                    nc.scalar.activation(wT[:, sl], zp[:],
                                         mybir.ActivationFunctionType.Sigmoid,
                                         bias=b2[:])
                for q in range(HPANEL):
                    gp = psum.tile([PANEL, D], FP32, tag="tps")
                    nc.tensor.transpose(gp[:], wT[:, q * PANEL:(q + 1) * PANEL],
                                        ident[:D, :D])
                    nc.scalar.mul(dtw[:, q * D:(q + 1) * D], gp[:], dt)

                # ---- scatter (one-hot matmuls); ACT drains PSUM -> Y -------
                # user panels read the item half of each pair and vice versa
                for p in range(NPANEL):
                    hoff = D if p < HPANEL else 0
                    ps = psum_s.tile([PANEL, D], FP32, tag="ps")
                    for k in range(K):
                        g, ci = int(gci[p, k, 0]), int(gci[p, k, 1])
                        c_lin = off[g] // CHUNK + ci
                        nc.tensor.matmul(
                            ps[:],
                            selm[:, c_lin * PANEL:(c_lin + 1) * PANEL],
                            G[g][:, ci * 2 * D + hoff:ci * 2 * D + hoff + D],
                            start=(k == 0), stop=(k == K - 1),
                        )
                    nc.scalar.copy(Ysb[:, p * D:(p + 1) * D], ps[:])

                # ---- batched Euler update (SBUF-only DVE, four quarters so
                # only the last quarter waits on the final scatter chunks);
                # quarters 0,1 = user panels, 2,3 = item panels -------------
                QB = NPANEL // 4 * D    # 256 cols = 4 panels
                for h in range(4):
                    sl = slice(h * QB, (h + 1) * QB)
                    dsl = slice((h % 2) * QB, (h % 2) * QB + QB)
                    eff = work.tile([PANEL, QB], FP32, tag="eff")
                    nc.vector.tensor_tensor(eff[:], Ysb[:, sl], Tcur[:, sl],
                                            op=mybir.AluOpType.subtract)
                    nc.vector.tensor_tensor(eff[:], eff[:], dtw[:, dsl],
                                            op=mybir.AluOpType.mult)
                    nc.vector.tensor_tensor(Tnxt[:, sl], Tcur[:, sl], eff[:],
                                            op=mybir.AluOpType.add)
                    if step < NSTEP - 1:
                        # pair-pack into the publish stage: user panels fill
                        # cols [0,D) of pair-panel 4h.., item panels [D,2D)
                        pbase = 4 * h if h < 2 else 4 * (h - 2)
                        coff = 0 if h < 2 else D
                        nc.vector.tensor_copy(
                            agstage[:].rearrange(
                                "j (p f) -> j p f",
                                f=2 * D)[:, pbase:pbase + 4, coff:coff + D],
                            Tnxt[:].rearrange("j (p f) -> j p f",
                                              f=D)[:, 4 * h:4 * h + 4, :])
                    elif h % 2 == 1:
                        ho = (h // 2) * (HPANEL * D)
                        nc.sync.dma_start(
                            outsl.ap()[:, ho:ho + HPANEL * D],
                            Tnxt[:, ho:ho + HPANEL * D])

                # ---- publish updated pair table ----------------------------
                if step < NSTEP - 1:
                    nc.sync.dma_start(
                        ag_in[step].ap().rearrange("(p j) f -> j p f",
                                                   j=PANEL),
                        agstage[:].rearrange("j (p f) -> j p f", f=2 * D))
                    nc.gpsimd.collective_compute(
                        "AllGather",
                        mybir.AluOpType.bypass,
                        replica_groups=[list(range(NCORES))],
                        ins=[ag_in[step].ap().opt()],
                        outs=[tbl_ag[step].ap().opt()],
                    )

    nc.compile()
    return nc


# ----------------------------------------------------------------------------
# Entry point
# ----------------------------------------------------------------------------

def kernel(users, items, user_emb, item_emb, w1, b1, w2, b2,
           edge_src, edge_dst, edge_vals, time_steps):
    users = np.asarray(users)
    items = np.asarray(items)
    user_emb = np.asarray(user_emb, dtype=np.float32)
    item_emb = np.asarray(item_emb, dtype=np.float32)
    w1 = np.asarray(w1, dtype=np.float32)
    b1 = np.asarray(b1, dtype=np.float32)
    w2 = np.asarray(w2, dtype=np.float32)
    b2 = np.asarray(b2, dtype=np.float32)
    time_steps = np.asarray(time_steps, dtype=np.float32)
    dts = np.diff(time_steps)

    # initial compact table
    E_u = user_emb[users]
    E_i = item_emb[items]
    bidx = np.arange(B)
    pos = _balance_pos(edge_src, edge_dst)
    rows_u = _compact_rows_user(bidx, pos)
    rows_i = _compact_rows_item(bidx, pos)
    table0 = np.zeros((ROWS, D), np.float32)
    table0[rows_u] = E_u
    table0[rows_i] = E_i
    # pair-packed table: row (1024c + k) = [user row | item row] of batch pair
    t3 = table0.reshape(NCORES, 2, HALF, D)
    table0p = np.concatenate([t3[:, 0], t3[:, 1]], axis=2).reshape(
        PAIRS, 2 * D).astype(BF16)

    K, nchunk, nslots, sel_bf, gidx, ncnt = _preprocess_edges(
        edge_src, edge_dst, edge_vals, pos)

    key = (K, nchunk, nslots, ncnt, tuple(np.round(dts, 9).tolist()))
    if key not in _PROG_CACHE:
        _PROG_CACHE[key] = _build_program(K, nchunk, nslots, dts, ncnt)
    nc = _PROG_CACHE[key]

    w1u = np.ascontiguousarray(w1[:D]).astype(BF16)
    w1i = np.ascontiguousarray(w1[D:]).astype(BF16)
    w2b = w2.astype(BF16)
    b1c = np.ascontiguousarray(b1.reshape(D, 1))
    b2c = np.ascontiguousarray(b2.reshape(D, 1))

    in_maps = []
    for c in range(NCORES):
        sl = table0[c * SLICE:(c + 1) * SLICE]
        in_maps.append({
            "table0": table0p,
            "slice0": _slice_layout(sl),
            "selm": sel_bf[c],
            "gidx": np.tile(gidx[c], (8, 1)),
            "w1u": w1u, "w1i": w1i, "w2": w2b, "b1": b1c, "b2": b2c,
        })

    res = bass_utils.run_bass_kernel_spmd(
        nc, in_maps, core_ids=list(range(NCORES)),
        trace=False)
    kernel.last_results = res

    final = np.zeros((ROWS, D), np.float32)
    for c in range(NCORES):
        final[c * SLICE:(c + 1) * SLICE] = _unslice_layout(
            res.results[c]["outslice"])

    Uf = final[rows_u]
    If = final[rows_i]
    logits = np.sum(Uf * If, axis=1)
    return (1.0 / (1.0 + np.exp(-logits))).astype(np.float32)


# revision 63
# speedup vs baseline: 1.0097x; 1.0097x over previous
"""Trainium2 Bass kernel for nn_CDECF (graph-ODE collaborative filtering).

Contract: kernel(**inputs) takes FULL unsharded numpy inputs (as produced by
reference.setup_inputs()) and returns the FULL [8192] float32 output.

Strategy
--------
The reference scatters the 8192 batch-user embeddings into node rows [0,B)
and batch-item embeddings into rows [NU, NU+B) of a 70000-row node tensor,
runs segment_sum over 2M edges, and reads back only those same rows. Hence
only edges with BOTH endpoints inside those two windows contribute; host
preprocessing compacts the problem to a 16384-row space (~134k edges).

Compact row layout: core c owns user rows for batch b in [1024c, 1024c+1024)
followed by the matching item rows, so the MLP gate is purely core-local.

The published table packs each batch pair into one 256B row:
row b = [user_b (64 bf16) | item_b (64 bf16)]. The graph is bipartite, so a
user-row output panel only ever reads item rows and vice versa — the scatter
matmul statically selects the needed half of each gathered element, and the
256B dma_gather granularity carries zero padding. One 256KB-in AllGather per
step boundary republishes the table.

Per ODE step, per core:
  - dma_gather the referenced pair rows (deduped per output panel; call-tail
    padding is skipped via negative indices + per-core runtime counts)
  - scatter-add via one-hot selection-matrix matmuls on the TensorEngine,
    K chunks of <=128 slots per 128-row output panel, accumulated in PSUM
  - gate MLP computed from the core-local fp32 slice (overlaps the gather)
  - Euler update in fp32; the pair-packed bf16 stage republishes via
    AllGather
Final scoring (sigmoid of U.I) is a trivial host-side epilogue.
"""
import sys

for _p in ("/opt/trn_rl_repo", "/root/.axon_site/_ro/trn_rl_repo"):
    if _p not in sys.path:
        sys.path.append(_p)

import numpy as np
import ml_dtypes

import concourse.bass as bass
import concourse.bacc as bacc
import concourse.mybir as mybir
import concourse.tile as tile
from concourse import bass_utils
from concourse.masks import make_identity

BF16 = ml_dtypes.bfloat16

NCORES = 8
NU, NI, B, D = 50000, 20000, 8192, 64
ROWS = 2 * B          # 16384 compact rows
SLICE = ROWS // NCORES  # 2048 rows per core
HALF = SLICE // 2     # 1024 batch pairs per core
PAIRS = NCORES * HALF  # 8192 pair rows in the published table
PANEL = 128
NPANEL = SLICE // PANEL  # 16 panels per core
HPANEL = NPANEL // 2     # 8 panels per half
CHUNK = 128           # slots per scatter matmul
GCALL = 1024          # gather idxs per dma_gather call (descriptor-ring cap)
NSTEP = 3

_PROG_CACHE = {}


# ----------------------------------------------------------------------------
# Host preprocessing
# ----------------------------------------------------------------------------

def _compact_rows_user(b, pos):
    return SLICE * (b // HALF) + pos[b]


def _compact_rows_item(b, pos):
    return SLICE * (b // HALF) + HALF + pos[b]


def _balance_pos(edge_src, edge_dst):
    """batch index -> row position within its core half (panel*128 + i).

    Shared by the user and item halves (the published table pairs batch b's
    user and item rows at one position). Chosen by a dedup-aware greedy so
    every output panel's distinct-source count stays under 8*128, i.e. the
    scatter needs K=8 chunks per panel instead of 9 (~11% less gather
    traffic, and 16 gather calls = 4 even SWDGE queue rounds).
    """
    src = np.asarray(edge_src).astype(np.int64)
    dst = np.asarray(edge_dst).astype(np.int64)

    def in_s(x):
        return (x < B) | ((x >= NU) & (x < NU + B))

    mask = in_s(src) & in_s(dst)
    s, d = src[mask], dst[mask]
    ob = np.where(s < B, s, s - NU)
    sb = np.where(d < B, d, d - NU)
    rowid = np.where(s < B, ob, B + ob)
    order = np.argsort(rowid * B + sb)
    r_s, s_s = rowid[order], sb[order]
    keep = np.ones(len(r_s), bool)
    keep[1:] = (r_s[1:] != r_s[:-1]) | (s_s[1:] != s_s[:-1])
    r_s, s_s = r_s[keep], s_s[keep]
    starts = np.searchsorted(r_s, np.arange(2 * B + 1))

    pos = np.zeros(B, np.int64)
    for c in range(NCORES):
        batches = list(range(c * HALF, (c + 1) * HALF))
        US = {b: s_s[starts[b]:starts[b + 1]].tolist() for b in batches}
        IS = {b: s_s[starts[B + b]:starts[B + b + 1]].tolist()
              for b in batches}
        batches.sort(key=lambda b: -(len(US[b]) + len(IS[b])))
        pu = [set() for _ in range(HPANEL)]
        pi = [set() for _ in range(HPANEL)]
        pc = [0] * HPANEL
        plist = [[] for _ in range(HPANEL)]
        for b in batches:
            su, si = US[b], IS[b]
            best, bestsc = None, None
            for p in range(HPANEL):
                if pc[p] >= PANEL:
                    continue
                ru = len(pu[p]) + sum(1 for x in su if x not in pu[p])
                ri = len(pi[p]) + sum(1 for x in si if x not in pi[p])
                sc = (max(ru, ri), ru + ri)
                if bestsc is None or sc < bestsc:
                    best, bestsc = p, sc
            pu[best].update(su)
            pi[best].update(si)
            pc[best] += 1
            plist[best].append(b)
        for p in range(HPANEL):
            for i, b in enumerate(plist[p]):
                pos[b] = p * PANEL + i
    return pos


def _chunk_gci(K):
    """chunk (panel, k) -> (gather call g, chunk-within-call ci).

    Per phase (user panels 0-7 / item panels 8-15) each panel's last chunk is
    the partially-filled one; place exactly one of them at each call's tail so
    padding is contiguous at the call end (skippable via negative indices).
    """
    cpc = GCALL // CHUNK
    gci = np.zeros((NPANEL, K, 2), np.int64)
    if K < cpc:
        for p in range(NPANEL):
            for k in range(K):
                c = p * K + k
                gci[p, k] = divmod(c, cpc)
        return gci
    for ph in range(2):
        panels = range(ph * HPANEL, (ph + 1) * HPANEL)
        fulls = [(p, k) for p in panels for k in range(K - 1)]
        partials = [(p, K - 1) for p in panels]
        fi = 0
        for gi in range(K):
            if gi < len(partials):
                mine = fulls[fi:fi + cpc - 1] + [partials[gi]]
                fi += cpc - 1
            else:
                mine = fulls[fi:fi + cpc]
                fi += cpc
            for ci, (p, k) in enumerate(mine):
                gci[p, k] = (ph * K + gi, ci)
        assert fi == len(fulls)
    return gci


def _preprocess_edges(edge_src, edge_dst, edge_vals, pos):
    src = np.asarray(edge_src).astype(np.int64)
    dst = np.asarray(edge_dst).astype(np.int64)
    val = np.asarray(edge_vals).astype(np.float32)

    def in_s(x):
        return (x < B) | ((x >= NU) & (x < NU + B))

    mask = in_s(src) & in_s(dst)
    s, d, v = src[mask], dst[mask], val[mask]

    def compact(ids):
        b = np.where(ids < B, ids, ids - NU)
        return (SLICE * (b // HALF) + pos[b]
                + np.where(ids < B, 0, HALF)).astype(np.int64)

    cs, cd = compact(s), compact(d)

    # bipartite invariant: user-row outputs read item rows and vice versa
    assert np.all(((cs % SLICE) < HALF) == ((cd % SLICE) >= HALF))

    pg = cs // PANEL                        # output panel 0..127
    pr = (cd // SLICE) * HALF + (cd % SLICE) % HALF   # source pair id
    rloc = cs % PANEL                       # row within output panel

    # dedup (panel, pair) -> one gather slot; multi-hot sel absorbs repeats
    skey = pg * PAIRS + pr
    uq, inv = np.unique(skey, return_inverse=True)
    upg = uq // PAIRS
    upr = uq % PAIRS
    counts = np.bincount(upg, minlength=ROWS // PANEL)
    K = int(np.ceil(counts.max() / CHUNK))
    nchunk = NPANEL * K
    nslots = nchunk * CHUNK
    ncall = nslots // GCALL
    assert nslots % GCALL == 0

    base = np.zeros(ROWS // PANEL, np.int64)
    base[1:] = np.cumsum(counts)[:-1]
    urank = np.arange(len(uq)) - base[upg]  # rank within panel (pr-sorted)
    ucore = upg // NPANEL
    up = upg % NPANEL
    k_u = urank // CHUNK
    pos_u = urank % CHUNK

    gci = _chunk_gci(K)
    g_u = gci[up, k_u, 0]
    ci_u = gci[up, k_u, 1]
    slot_u = g_u * GCALL + ci_u * CHUNK + pos_u

    idx_arr = np.full((NCORES, nslots), -1, np.int16)
    idx_arr[ucore, slot_u] = upr.astype(np.int16)

    sel = np.zeros((NCORES, nslots, PANEL), np.float32)
    np.add.at(sel, (ucore[inv], slot_u[inv], rloc), v)
    # SBUF layout [core, 128 slot-partitions, nchunk*128 row-cols], with
    # columns ordered by linear chunk id c_lin = g*cpc + ci
    sel = sel.reshape(NCORES, nchunk, CHUNK, PANEL).transpose(0, 2, 1, 3)
    sel = np.ascontiguousarray(sel.reshape(NCORES, CHUNK, nchunk * PANEL))
    sel_bf = sel.astype(BF16)

    # per-(core, call): negatives are only allowed at the call tail; convert
    # interior holes to 0 (harmless: sel there is 0). The per-call slot count
    # must be identical across cores (it is baked into the SPMD program), so
    # pad every core up to the max core's count with index 0 and put -1 only
    # beyond it.
    nreal = np.zeros((NCORES, ncall), np.int32)
    for c in range(NCORES):
        for g in range(ncall):
            blk = idx_arr[c, g * GCALL:(g + 1) * GCALL]
            real = np.nonzero(blk >= 0)[0]
            last = real[-1]
            blk[:last + 1][blk[:last + 1] < 0] = 0
            nreal[c, g] = np.count_nonzero(blk >= 0)
    ncnt = nreal.max(axis=0).astype(np.int64)   # per-call count, all cores
    for c in range(NCORES):
        for g in range(ncall):
            blk = idx_arr[c, g * GCALL:(g + 1) * GCALL]
            blk[nreal[c, g]:ncnt[g]] = 0

    # wrapped gather indices: per call block of GCALL slots, wrapped into
    # 16 partitions: wrapped[p, s] = block_idx[s*16 + p]
    w = idx_arr.reshape(NCORES, ncall, GCALL // 16, 16).transpose(0, 3, 1, 2)
    gidx = np.ascontiguousarray(w.reshape(NCORES, 16, ncall * (GCALL // 16)))

    return K, nchunk, nslots, sel_bf, gidx, tuple(int(x) for x in ncnt)


def _slice_layout(slice_2d):
    """[2048, 64] -> SBUF layout [128, 16*64] (partition = row-in-panel)."""
    return np.ascontiguousarray(
        slice_2d.reshape(NPANEL, PANEL, D).transpose(1, 0, 2).reshape(PANEL,
                                                                      NPANEL * D))


def _unslice_layout(arr):
    """[128, 16*64] -> [2048, 64]."""
    return arr.reshape(PANEL, NPANEL, D).transpose(1, 0, 2).reshape(SLICE, D)


# ----------------------------------------------------------------------------
# Device program
# ----------------------------------------------------------------------------

def _build_program(K, nchunk, nslots, dts, ncnt):
    FP32 = mybir.dt.float32
    BF = mybir.dt.bfloat16
    nc = bacc.Bacc("TRN2", target_bir_lowering=False, debug=False,
                   num_devices=NCORES, num_swdge_queues=4)

    ncall = nslots // GCALL
    cpc = GCALL // CHUNK
    gci = _chunk_gci(K)

    # --- I/O -----------------------------------------------------------------
    table0 = nc.dram_tensor("table0", [PAIRS, 2 * D], BF, kind="ExternalInput")
    slice0 = nc.dram_tensor("slice0", [PANEL, NPANEL * D], FP32,
                            kind="ExternalInput")
    selm_in = nc.dram_tensor("selm", [PANEL, nchunk * PANEL], BF,
                             kind="ExternalInput")
    gidx_in = nc.dram_tensor("gidx", [128, nslots // 16], mybir.dt.int16,
                             kind="ExternalInput")
    w1u_in = nc.dram_tensor("w1u", [D, D], BF, kind="ExternalInput")
    w1i_in = nc.dram_tensor("w1i", [D, D], BF, kind="ExternalInput")
    w2_in = nc.dram_tensor("w2", [D, D], BF, kind="ExternalInput")
    b1_in = nc.dram_tensor("b1", [D, 1], FP32, kind="ExternalInput")
    b2_in = nc.dram_tensor("b2", [D, 1], FP32, kind="ExternalInput")
    outsl = nc.dram_tensor("outslice", [PANEL, NPANEL * D], FP32,
                           kind="ExternalOutput")

    # --- internal DRAM -------------------------------------------------------
    ag_in = [nc.dram_tensor(f"ag_in{s}", [HALF, 2 * D], BF)
             for s in range(NSTEP - 1)]
    tbl_ag = [nc.dram_tensor(f"tbl_ag{s}", [PAIRS, 2 * D], BF,
                             addr_space="Shared") for s in range(NSTEP - 1)]
    warm_in = nc.dram_tensor("warm_in", [16, 16], BF)
    warm_out = nc.dram_tensor("warm_out", [128, 16], BF, addr_space="Shared")

    with tile.TileContext(nc) as tc:
        with (
            tc.tile_pool(name="cst", bufs=1) as cst,
            tc.tile_pool(name="state", bufs=1) as state,
            tc.tile_pool(name="work", bufs=2) as work,
            tc.tile_pool(name="psum", bufs=2, space="PSUM") as psum,
            tc.tile_pool(name="psum_s", bufs=3, space="PSUM") as psum_s,
        ):
            # --- persistent tiles -------------------------------------------
            selm = cst.tile([PANEL, nchunk * PANEL], BF)
            gidx = cst.tile([128, nslots // 16], mybir.dt.int16)
            w1u = cst.tile([D, D], BF)
            w1i = cst.tile([D, D], BF)
            w2 = cst.tile([D, D], BF)
            b1 = cst.tile([D, 1], FP32)
            b2 = cst.tile([D, 1], FP32)
            ident = cst.tile([PANEL, PANEL], FP32)
            T = [state.tile([PANEL, NPANEL * D], FP32, name=f"T{i}")
                 for i in range(2)]
            G = [state.tile([PANEL, GCALL], BF, name=f"G{g}")
                 for g in range(ncall)]
            agstage = state.tile([PANEL, HPANEL * 2 * D], BF)
            xTu = state.tile([D, HPANEL * PANEL], BF)
            xTi = state.tile([D, HPANEL * PANEL], BF)
            hT = state.tile([D, HPANEL * PANEL], BF)
            wT = state.tile([D, HPANEL * PANEL], FP32)
            dtw = state.tile([PANEL, HPANEL * D], FP32)
            Ysb = state.tile([PANEL, NPANEL * D], FP32)

            # gidx first: the step-0 gathers wait only on it (table0 is an
            # ExternalInput already resident in HBM); selm isn't needed until
            # the first scatter, so it loads behind the gathers.
            nc.sync.dma_start(gidx[:], gidx_in[:])
            nc.sync.dma_start(w1u[:], w1u_in[:])
            nc.sync.dma_start(w1i[:], w1i_in[:])
            nc.sync.dma_start(w2[:], w2_in[:])
            nc.sync.dma_start(b1[:], b1_in[:])
            nc.sync.dma_start(b2[:], b2_in[:])
            nc.sync.dma_start(T[0][:], slice0[:])
            nc.sync.dma_start(selm[:], selm_in[:])
            nc.vector.memset(agstage[:], 0.0)
            # skipped call-tail slots leave G untouched; zero once so the
            # (sel=0-masked) stale columns are finite
            for g in range(ncall):
                nc.vector.memset(G[g][:], 0.0)
            make_identity(nc, ident[:])

            def emit_warmup():
                # warm the ncfw collective path under the step-0 gathers so
                # the first real AllGather doesn't pay the cold trigger delay
                nc.gpsimd.collective_compute(
                    "AllGather",
                    mybir.AluOpType.bypass,
                    replica_groups=[list(range(NCORES))],
                    ins=[warm_in.ap().opt()],
                    outs=[warm_out.ap().opt()],
                )

            emit_warmup()

            for step in range(NSTEP):
                dt = float(dts[step])
                Tcur = T[step % 2]
                Tnxt = T[(step + 1) % 2]
                tbl = table0 if step == 0 else tbl_ag[step - 1]

                # ---- gather referenced pair rows ---------------------------
                for g in range(ncall):
                    nc.gpsimd.dma_gather(
                        out_ap=G[g][:].rearrange("p (c e) -> p c e", e=2 * D),
                        in_ap=tbl.ap(),
                        idxs_ap=gidx[:, g * (GCALL // 16):(g + 1) * (GCALL // 16)],
                        num_idxs=GCALL,
                        num_idxs_reg=int(ncnt[g]),
                        elem_size=2 * D,
                        queue_num=g % 4,
                    )

                # ---- gate MLP from local fp32 slice (core-local) -----------
                for p in range(NPANEL):
                    tp = psum.tile([D, PANEL], FP32, tag="tps")
                    nc.tensor.transpose(tp[:], Tcur[:, p * D:(p + 1) * D],
                                        ident[:])
                    dst = xTu if p < HPANEL else xTi
                    q = p % HPANEL
                    nc.scalar.copy(dst[:, q * PANEL:(q + 1) * PANEL], tp[:])
                for chix in range(2):
                    sl = slice(chix * 512, (chix + 1) * 512)
                    hp = psum.tile([D, 512], FP32, tag="mlp")
                    nc.tensor.matmul(hp[:], w1u[:], xTu[:, sl],
                                     start=True, stop=False)
                    nc.tensor.matmul(hp[:], w1i[:], xTi[:, sl],
                                     start=False, stop=True)
                    nc.scalar.activation(hT[:, sl], hp[:],
                                         mybir.ActivationFunctionType.Relu,
                                         bias=b1[:])
                    zp = psum.tile([D, 512], FP32, tag="mlp")
                    nc.tensor.matmul(zp[:], w2[:], hT[:, sl],
                                     start=True, stop=True)
                    nc.scalar.activation(wT[:, sl], zp[:],
                                         mybir.ActivationFunctionType.Sigmoid,
                                         bias=b2[:])
                for q in range(HPANEL):
                    gp = psum.tile([PANEL, D], FP32, tag="tps")
                    nc.tensor.transpose(gp[:], wT[:, q * PANEL:(q + 1) * PANEL],
                                        ident[:D, :D])
                    nc.scalar.mul(dtw[:, q * D:(q + 1) * D], gp[:], dt)

                # ---- scatter (one-hot matmuls); ACT drains PSUM -> Y -------
                # user panels read the item half of each pair and vice versa
                for p in range(NPANEL):
                    off = D if p < HPANEL else 0
                    ps = psum_s.tile([PANEL, D], FP32, tag="ps")
                    for k in range(K):
                        g, ci = int(gci[p, k, 0]), int(gci[p, k, 1])
                        c_lin = g * cpc + ci
                        nc.tensor.matmul(
                            ps[:],
                            selm[:, c_lin * PANEL:(c_lin + 1) * PANEL],
                            G[g][:, ci * 2 * D + off:ci * 2 * D + off + D],
                            start=(k == 0), stop=(k == K - 1),
                        )
                    nc.scalar.copy(Ysb[:, p * D:(p + 1) * D], ps[:])

                # ---- batched Euler update (SBUF-only DVE, four quarters so
                # only the last quarter waits on the final scatter chunks);
                # quarters 0,1 = user panels, 2,3 = item panels -------------
                QB = NPANEL // 4 * D    # 256 cols = 4 panels
                for h in range(4):
                    sl = slice(h * QB, (h + 1) * QB)
                    dsl = slice((h % 2) * QB, (h % 2) * QB + QB)
                    eff = work.tile([PANEL, QB], FP32, tag="eff")
                    nc.vector.tensor_tensor(eff[:], Ysb[:, sl], Tcur[:, sl],
                                            op=mybir.AluOpType.subtract)
                    nc.vector.tensor_tensor(eff[:], eff[:], dtw[:, dsl],
                                            op=mybir.AluOpType.mult)
                    nc.vector.tensor_tensor(Tnxt[:, sl], Tcur[:, sl], eff[:],
                                            op=mybir.AluOpType.add)
                    if step < NSTEP - 1:
                        # pair-pack into the publish stage: user panels fill
                        # cols [0,D) of pair-panel 4h.., item panels [D,2D)
                        pbase = 4 * h if h < 2 else 4 * (h - 2)
                        coff = 0 if h < 2 else D
                        nc.vector.tensor_copy(
                            agstage[:].rearrange(
                                "j (p f) -> j p f",
                                f=2 * D)[:, pbase:pbase + 4, coff:coff + D],
                            Tnxt[:].rearrange("j (p f) -> j p f",
                                              f=D)[:, 4 * h:4 * h + 4, :])
                    elif h % 2 == 1:
                        ho = (h // 2) * (HPANEL * D)
                        nc.sync.dma_start(
                            outsl.ap()[:, ho:ho + HPANEL * D],
                            Tnxt[:, ho:ho + HPANEL * D])

                # ---- publish updated pair table ----------------------------
                if step < NSTEP - 1:
                    nc.sync.dma_start(
                        ag_in[step].ap().rearrange("(p j) f -> j p f",
                                                   j=PANEL),
                        agstage[:].rearrange("j (p f) -> j p f", f=2 * D))
                    nc.gpsimd.collective_compute(
                        "AllGather",
                        mybir.AluOpType.bypass,
                        replica_groups=[list(range(NCORES))],
                        ins=[ag_in[step].ap().opt()],
                        outs=[tbl_ag[step].ap().opt()],
                    )

    nc.compile()
    return nc


# ----------------------------------------------------------------------------
# Entry point
# ----------------------------------------------------------------------------

def kernel(users, items, user_emb, item_emb, w1, b1, w2, b2,
           edge_src, edge_dst, edge_vals, time_steps):
    users = np.asarray(users)
    items = np.asarray(items)
    user_emb = np.asarray(user_emb, dtype=np.float32)
    item_emb = np.asarray(item_emb, dtype=np.float32)
    w1 = np.asarray(w1, dtype=np.float32)
    b1 = np.asarray(b1, dtype=np.float32)
    w2 = np.asarray(w2, dtype=np.float32)
    b2 = np.asarray(b2, dtype=np.float32)
    time_steps = np.asarray(time_steps, dtype=np.float32)
    dts = np.diff(time_steps)

    # initial compact table
    E_u = user_emb[users]
    E_i = item_emb[items]
    bidx = np.arange(B)
    pos = _balance_pos(edge_src, edge_dst)
    rows_u = _compact_rows_user(bidx, pos)
    rows_i = _compact_rows_item(bidx, pos)
    table0 = np.zeros((ROWS, D), np.float32)
    table0[rows_u] = E_u
    table0[rows_i] = E_i
    # pair-packed table: row (1024c + k) = [user row | item row] of batch pair
    t3 = table0.reshape(NCORES, 2, HALF, D)
    table0p = np.concatenate([t3[:, 0], t3[:, 1]], axis=2).reshape(
        PAIRS, 2 * D).astype(BF16)

    K, nchunk, nslots, sel_bf, gidx, ncnt = _preprocess_edges(
        edge_src, edge_dst, edge_vals, pos)

    key = (K, nchunk, nslots, ncnt, tuple(np.round(dts, 9).tolist()))
    if key not in _PROG_CACHE:
        _PROG_CACHE[key] = _build_program(K, nchunk, nslots, dts, ncnt)
    nc = _PROG_CACHE[key]

    w1u = np.ascontiguousarray(w1[:D]).astype(BF16)
    w1i = np.ascontiguousarray(w1[D:]).astype(BF16)
    w2b = w2.astype(BF16)
    b1c = np.ascontiguousarray(b1.reshape(D, 1))
    b2c = np.ascontiguousarray(b2.reshape(D, 1))

    in_maps = []
    for c in range(NCORES):
        sl = table0[c * SLICE:(c + 1) * SLICE]
        in_maps.append({
            "table0": table0p,
            "slice0": _slice_layout(sl),
            "selm": sel_bf[c],
            "gidx": np.tile(gidx[c], (8, 1)),
            "w1u": w1u, "w1i": w1i, "w2": w2b, "b1": b1c, "b2": b2c,
        })

    res = bass_utils.run_bass_kernel_spmd(
        nc, in_maps, core_ids=list(range(NCORES)),
        trace=False)
    kernel.last_results = res

    final = np.zeros((ROWS, D), np.float32)
    for c in range(NCORES):
        final[c * SLICE:(c + 1) * SLICE] = _unslice_layout(
            res.results[c]["outslice"])

    Uf = final[rows_u]
    If = final[rows_i]
    logits = np.sum(Uf * If, axis=1)
    return (1.0 / (1.0 + np.exp(-logits))).astype(np.float32)
